# revision 44
# baseline (speedup 1.0000x reference)
"""Trainium2 Bass kernel for: y = k*tanh(x@w/d + b)[:,None] * w[None,:] + c*x.

Data-parallel over 8 NeuronCores: x is [16384, 4096] f32, sharded 2048
rows/core; w/c/k/b are tiny and folded host-side:
  wd = w/d            (dot-product weights; /d folded in)
  kw = k*w            (outer-product weights; k folded in)
  b  -> tanh bias
  c  -> if c != 1: feed x' = c*x and wd' = w/(d*c); identity otherwise.

Per-core device program (16 tiles of [128 rows, 4096 cols], bf16 I/O):
  DMA in x_tile
  dot  = sum(x * wd) per row     (one fused DVE pass, or DVE mult + ACT accum)
  h    = tanh(dot + b)           (ACT)
  y    = kw * h + c*x            (DVE tensor_scalar + add, or TensorE PSUM
                                  accumulate: identity-matmul c*x + rank-1
                                  h (x) kw, with ACT PSUM->SBUF copy-out)
  DMA out y_tile

Memory-bound: ~32 MB HBM traffic/core at ~358 GB/s. bf16 I/O halves DMA
traffic and doubles DVE throughput at ~0.3% output error (well under the
tolerance); h is computed via an f32 accumulator.
"""

import os

import numpy as np

B = 16384
D = 4096
N_CORES = 8
P = 128
B_SHARD = B // N_CORES          # 2048 rows per core
N_TILES = B_SHARD // P          # 16 tiles per core

# variant knobs (resolved at import; see _build)
DTYPE = os.environ.get("NK_DTYPE", "bf16")        # "f32" | "bf16"
DOT = os.environ.get("NK_DOT", "tt_act")          # "stt" | "tt_act"
COMB = os.environ.get("NK_COMB", "split")         # "stt" | "split" | "split_gp" | "pe"
GP_COLS = int(os.environ.get("NK_GP_COLS", "0"))  # cols of final add on gpsimd
XBUFS = int(os.environ.get("NK_XBUFS", "8"))
YBUFS = int(os.environ.get("NK_YBUFS", "3"))
TSBUFS = int(os.environ.get("NK_TSBUFS", "2"))
CONST_BCAST = os.environ.get("NK_CONST_BCAST", "pe")  # "0" | "gp" | "pe"
ODMA = os.environ.get("NK_ODMA", "sync")          # engine issuing out-DMAs
IDMA = os.environ.get("NK_IDMA", "sync")          # engine issuing x in-DMAs

_CACHE = {}


def _build(add_x, b_val, dtype=DTYPE, dot=DOT, comb=COMB, gp_cols=GP_COLS,
           xbufs=XBUFS, ybufs=YBUFS, n_tiles=N_TILES, tsbufs=TSBUFS,
           const_bcast=CONST_BCAST, odma=ODMA):
    """Build + compile the per-core Bass program (SPMD, same graph on all cores)."""
    from contextlib import ExitStack

    import concourse.bass as bass  # noqa: F401
    import concourse.tile as tile
    from concourse import bacc, mybir

    f32 = mybir.dt.float32
    dt = mybir.dt.bfloat16 if dtype == "bf16" else f32
    rows = n_tiles * P

    nc = bacc.Bacc(
        "TRN2",
        debug=False,
        target_bir_lowering=False,
        num_devices=N_CORES,
    )

    # wd: full [P, D] direct DMA unless a broadcast mode covers it ("kwpe"
    # broadcasts only kw and DMAs wd directly — wd gates the first dot pass,
    # and the DMA lands sooner than the PE-broadcast chain completes).
    if comb == "pe2":
        # wd direct [P, D] DMA (frees all PSUM for the combine); kw as a row.
        wd_p, kw_p = P, 1
    else:
        wd_p = P if const_bcast in ("0", "kwpe") else 1
        kw_p = P if const_bcast == "0" else 1
    x_ext = nc.dram_tensor("x", [rows, D], dt, kind="ExternalInput").ap()
    wd_ext = nc.dram_tensor("wd", [wd_p, D], dt, kind="ExternalInput").ap()
    kw_ext = nc.dram_tensor("kw", [kw_p, D], dt, kind="ExternalInput").ap()
    y_ext = nc.dram_tensor("y", [rows, D], dt, kind="ExternalOutput").ap()

    mult = mybir.AluOpType.mult
    add = mybir.AluOpType.add

    with tile.TileContext(nc) as tc, ExitStack() as ctx:
        consts = ctx.enter_context(tc.tile_pool(name="consts", bufs=1))
        xs = ctx.enter_context(tc.tile_pool(name="xs", bufs=xbufs))
        ys = ctx.enter_context(tc.tile_pool(name="ys", bufs=ybufs))
        ts = ctx.enter_context(tc.tile_pool(name="ts", bufs=tsbufs))
        ss = ctx.enter_context(tc.tile_pool(name="ss", bufs=int(os.environ.get("NK_SSBUFS", "4"))))

        odma_eng = getattr(nc, odma)
        idma_eng = getattr(nc, IDMA)
        bias_t = None

        def make_bias():
            t = consts.tile([P, 1], f32, tag="bias")
            nc.gpsimd.memset(t[:, :], float(b_val))
            return t

        if comb == "pe2":
            # v2 of the TensorE combine: DVE does only the fused dot;
            # TensorE accumulates c*x (identity matmul) + h (x) kw (K=1
            # rank-1 matmul) into [P, 2048] PSUM chunks (4 banks, bufs=2 =
            # all 8 banks); ACT copies PSUM->SBUF. wd arrives as a direct
            # [P, D] DMA; kw stays a [1, D] row.
            assert add_x
            wd_t = consts.tile([P, D], dt)
            kw_row = consts.tile([1, D], dt, tag="kw_row")
            nc.sync.dma_start(out=wd_t[:, :], in_=wd_ext[:, :])
            nc.sync.dma_start(out=kw_row[:, :], in_=kw_ext[:, :])
            # identity (c folded host-side; c==1 here)
            ci_t = consts.tile([P, P], dt, tag="ci")
            iota_t = consts.tile([P, P], f32, tag="iota")
            nc.gpsimd.iota(
                iota_t[:, :], [[1, P]], channel_multiplier=-1,
                allow_small_or_imprecise_dtypes=True,
            )
            nc.vector.tensor_scalar(
                out=ci_t[:, :], in0=iota_t[:, :], scalar1=0.0, scalar2=None,
                op0=mybir.AluOpType.is_equal,
            )
            bias_t = make_bias()

            yps = ctx.enter_context(tc.tile_pool(name="yps", bufs=2, space="PSUM"))
            hs = ctx.enter_context(tc.tile_pool(name="hs", bufs=4))

            def dot_pass_pe(x_t):
                trash = ts.tile([P, D], dt)
                dotv = ss.tile([P, 1], f32)
                nc.vector.scalar_tensor_tensor(
                    out=trash[:, :], in0=x_t[:, :], scalar=1.0, in1=wd_t[:, :],
                    op0=mult, op1=mult, accum_out=dotv[:, :],
                )
                h = hs.tile([P, 1], dt, tag="h")
                nc.scalar.activation(
                    h[:, :], dotv[:, :], mybir.ActivationFunctionType.Tanh,
                    bias=bias_t[:, :], scale=1.0,
                )
                # transpose h [P,1] -> hT [1,P] with a tiny SBUF->SBUF DMA
                hT = hs.tile([1, P], dt, tag="hT")
                nc.sync.dma_start(out=hT[0:1, :], in_=h[:, 0:1])
                return hT

            def combine_pe(x_t, hT, r0):
                y_t = ys.tile([P, D], dt)
                for cchunk in range(D // 2048):
                    cs = cchunk * 2048
                    ps = yps.tile([P, 2048], f32, tag="y")
                    for s in range(4):
                        a = cs + s * 512
                        nc.tensor.matmul(ps[:, s * 512 : (s + 1) * 512],
                                         ci_t[:, :], x_t[:, a : a + 512],
                                         start=True, stop=False)
                    for s in range(4):
                        a = cs + s * 512
                        nc.tensor.matmul(ps[:, s * 512 : (s + 1) * 512],
                                         hT[:, :], kw_row[:, a : a + 512],
                                         start=False, stop=True)
                    nc.scalar.copy(y_t[:, cs : cs + 2048], ps[:, :])
                odma_eng.dma_start(out=y_ext[r0 : r0 + P, :], in_=y_t[:, :])

            prev = None
            for i in range(n_tiles):
                r0 = i * P
                x_t = xs.tile([P, D], dt)
                idma_eng.dma_start(out=x_t[:, :], in_=x_ext[r0 : r0 + P, :])
                hT = dot_pass_pe(x_t)
                if prev is not None:
                    combine_pe(*prev)
                prev = (x_t, hT, r0)
            combine_pe(*prev)

        elif comb == "pe":
            # DVE does only the fused dot; TensorE accumulates c*x + h (x) kw
            # in PSUM (identity matmul + K=1 rank-1 matmul); ACT copies
            # PSUM->SBUF. kw stays a [1, D] row (rank-1 rhs); wd is broadcast
            # to [P, D] via a ones-matmul on the otherwise idle TensorE.
            assert const_bcast == "pe" and add_x
            wd_t = consts.tile([P, D], dt)
            wd_row = consts.tile([1, D], dt, tag="wd_row")
            kw_row = consts.tile([1, D], dt, tag="kw_row")
            ones_t = consts.tile([1, P], dt, tag="ones")
            nc.sync.dma_start(out=wd_row[:, :], in_=wd_ext[:, :])
            nc.sync.dma_start(out=kw_row[:, :], in_=kw_ext[:, :])
            nc.gpsimd.memset(ones_t[:, :], 1.0)
            bc_psum = ctx.enter_context(
                tc.tile_pool(name="bc_psum", bufs=2, space="PSUM"))
            for cchunk in range(D // 512):
                cs = cchunk * 512
                ps = bc_psum.tile([P, 512], f32, tag="bc")
                nc.tensor.matmul(
                    ps[:, :], ones_t[:, :], wd_row[:, cs : cs + 512],
                    start=True, stop=True,
                )
                nc.scalar.copy(wd_t[:, cs : cs + 512], ps[:, :])
            # identity (times c, folded host-side into x already; c==1 here)
            ci_t = consts.tile([P, P], dt, tag="ci")
            iota_t = consts.tile([P, P], f32, tag="iota")
            nc.gpsimd.iota(
                iota_t[:, :], [[1, P]], channel_multiplier=-1,
                allow_small_or_imprecise_dtypes=True,
            )
            nc.vector.tensor_scalar(
                out=ci_t[:, :], in0=iota_t[:, :], scalar1=0.0, scalar2=None,
                op0=mybir.AluOpType.is_equal,
            )
            bias_t = make_bias()

            yps = ctx.enter_context(tc.tile_pool(name="yps", bufs=3, space="PSUM"))
            hs = ctx.enter_context(tc.tile_pool(name="hs", bufs=4))

            def dot_pass_pe(x_t):
                trash = ts.tile([P, D], dt)
                dotv = ss.tile([P, 1], f32)
                nc.vector.scalar_tensor_tensor(
                    out=trash[:, :], in0=x_t[:, :], scalar=1.0, in1=wd_t[:, :],
                    op0=mult, op1=mult, accum_out=dotv[:, :],
                )
                h = hs.tile([P, 1], dt, tag="h")
                nc.scalar.activation(
                    h[:, :], dotv[:, :], mybir.ActivationFunctionType.Tanh,
                    bias=bias_t[:, :], scale=1.0,
                )
                # transpose h [P,1] -> hT [1,P] with a tiny SBUF->SBUF DMA
                hT = hs.tile([1, P], dt, tag="hT")
                nc.sync.dma_start(out=hT[0:1, :], in_=h[:, 0:1])
                return hT

            def combine_pe(x_t, hT, r0):
                y_t = ys.tile([P, D], dt)
                pss = []
                for cchunk in range(D // 1024):
                    cs = cchunk * 1024
                    ps = yps.tile([P, 1024], f32, tag="y")
                    nc.tensor.matmul(ps[:, 0:512], ci_t[:, :],
                                     x_t[:, cs : cs + 512], start=True, stop=False)
                    nc.tensor.matmul(ps[:, 512:1024], ci_t[:, :],
                                     x_t[:, cs + 512 : cs + 1024], start=True, stop=False)
                    pss.append((cs, ps))
                for cs, ps in pss:
                    nc.tensor.matmul(ps[:, 0:512], hT[:, :],
                                     kw_row[:, cs : cs + 512], start=False, stop=True)
                    nc.tensor.matmul(ps[:, 512:1024], hT[:, :],
                                     kw_row[:, cs + 512 : cs + 1024], start=False, stop=True)
                for cs, ps in pss:
                    nc.scalar.copy(y_t[:, cs : cs + 1024], ps[:, :])
                odma_eng.dma_start(out=y_ext[r0 : r0 + P, :], in_=y_t[:, :])

            prev = None
            for i in range(n_tiles):
                r0 = i * P
                x_t = xs.tile([P, D], dt)
                idma_eng.dma_start(out=x_t[:, :], in_=x_ext[r0 : r0 + P, :])
                hT = dot_pass_pe(x_t)
                if prev is not None:
                    combine_pe(*prev)
                prev = (x_t, hT, r0)
            combine_pe(*prev)

        else:
            wd_t = consts.tile([P, D], dt)
            kw_t = consts.tile([P, D], dt)
            if const_bcast == "gp":
                wd_row = consts.tile([1, D], dt, tag="wd_row")
                kw_row = consts.tile([1, D], dt, tag="kw_row")
                nc.sync.dma_start(out=wd_row[:, :], in_=wd_ext[:, :])
                nc.sync.dma_start(out=kw_row[:, :], in_=kw_ext[:, :])
                nc.gpsimd.partition_broadcast(wd_t[:, :], wd_row[:, :])
                nc.gpsimd.partition_broadcast(kw_t[:, :], kw_row[:, :])
            elif const_bcast in ("pe", "kwpe"):
                kw_row = consts.tile([1, D], dt, tag="kw_row")
                ones_t = consts.tile([1, P], dt, tag="ones")
                if const_bcast == "kwpe":
                    nc.sync.dma_start(out=wd_t[:, :], in_=wd_ext[:, :])
                else:
                    wd_row = consts.tile([1, D], dt, tag="wd_row")
                    nc.sync.dma_start(out=wd_row[:, :], in_=wd_ext[:, :])
                nc.sync.dma_start(out=kw_row[:, :], in_=kw_ext[:, :])
                nc.gpsimd.memset(ones_t[:, :], 1.0)
                psum = ctx.enter_context(
                    tc.tile_pool(name="bc_psum", bufs=2, space="PSUM"))

                bc_w = int(os.environ.get("NK_BC_W", "1024"))

                def bcast(row, dst, split_engines=False):
                    # bc_w-wide chunks: bc_w/512 matmuls (one PSUM bank each)
                    # + 1 PSUM->SBUF copy. split_engines alternates the copies
                    # between ACT and DVE (measured worse — keep off).
                    for cchunk in range(D // bc_w):
                        cs = cchunk * bc_w
                        ps = psum.tile([P, bc_w], f32, tag="bc")
                        for s in range(bc_w // 512):
                            a = cs + s * 512
                            nc.tensor.matmul(
                                ps[:, s * 512 : (s + 1) * 512], ones_t[:, :],
                                row[:, a : a + 512], start=True, stop=True,
                            )
                        if split_engines and cchunk % 2 == 1:
                            nc.vector.tensor_copy(dst[:, cs : cs + bc_w], ps[:, :])
                        else:
                            nc.scalar.copy(dst[:, cs : cs + bc_w], ps[:, :])

                # wd is needed by the very first dot pass -> broadcast it now
                # (unless it came via direct DMA in "kwpe" mode); kw is first
                # needed ~10us later and is broadcast after tile 0's dot pass.
                if const_bcast == "pe":
                    bcast(wd_row, wd_t)
            else:
                nc.sync.dma_start(out=wd_t[:, :], in_=wd_ext[:, :])
                nc.sync.dma_start(out=kw_t[:, :], in_=kw_ext[:, :])
            bias_t = make_bias()

            def dot_pass(x_t):
                dotv = ss.tile([P, 1], f32)
                if dot == "stt":
                    trash = ts.tile([P, D], dt)
                    nc.vector.scalar_tensor_tensor(
                        out=trash[:, :], in0=x_t[:, :], scalar=1.0, in1=wd_t[:, :],
                        op0=mult, op1=mult, accum_out=dotv[:, :],
                    )
                else:  # tt_act: DVE multiply, ACT accumulate-copy
                    t1 = ts.tile([P, D], dt)
                    nc.vector.tensor_mul(t1[:, :], x_t[:, :], wd_t[:, :])
                    t2 = ts.tile([P, D], dt, tag="t2")
                    nc.scalar.activation(
                        t2[:, :], t1[:, :], mybir.ActivationFunctionType.Copy,
                        accum_out=dotv[:, :],
                    )
                h = ss.tile([P, 1], f32)
                nc.scalar.activation(
                    h[:, :], dotv[:, :], mybir.ActivationFunctionType.Tanh,
                    bias=bias_t[:, :], scale=1.0,
                )
                return h

            def combine(x_t, h, r0):
                y_t = ys.tile([P, D], dt)
                if comb == "stt":
                    nc.vector.scalar_tensor_tensor(
                        out=y_t[:, :], in0=kw_t[:, :], scalar=h[:, :], in1=x_t[:, :],
                        op0=mult, op1=add if add_x else mybir.AluOpType.bypass,
                    )
                elif comb == "split_ip":
                    # in-place: y = kw*h, then y += x (no y1 buffer)
                    nc.vector.tensor_scalar(
                        out=y_t[:, :], in0=kw_t[:, :], scalar1=h[:, :], scalar2=None,
                        op0=mult,
                    )
                    if add_x:
                        nc.vector.tensor_add(y_t[:, :], y_t[:, :], x_t[:, :])
                else:
                    y1 = ts.tile([P, D], dt, tag="y1")
                    act_cols = int(os.environ.get("NK_ACT_COLS", "0"))
                    if comb == "split_act" and act_cols > 0:
                        # ACT computes a column-slice of kw*h via
                        # out = Copy(in*scale) with per-partition scale=h,
                        # offloading part of the DVE tensor_scalar.
                        nc.scalar.activation(
                            y1[:, :act_cols], kw_t[:, :act_cols],
                            mybir.ActivationFunctionType.Copy,
                            bias=0.0, scale=h[:, :],
                        )
                        nc.vector.tensor_scalar(
                            out=y1[:, act_cols:], in0=kw_t[:, act_cols:],
                            scalar1=h[:, :], scalar2=None, op0=mult,
                        )
                    else:
                        nc.vector.tensor_scalar(
                            out=y1[:, :], in0=kw_t[:, :], scalar1=h[:, :],
                            scalar2=None, op0=mult,
                        )
                    if not add_x:
                        y_t = y1
                    elif comb == "split_gp" and gp_cols > 0:
                        cs = D - gp_cols
                        nc.vector.tensor_add(y_t[:, :cs], y1[:, :cs], x_t[:, :cs])
                        nc.gpsimd.tensor_add(y_t[:, cs:], y1[:, cs:], x_t[:, cs:])
                    else:
                        nc.vector.tensor_add(y_t[:, :], y1[:, :], x_t[:, :])
                odma_eng.dma_start(out=y_ext[r0 : r0 + P, :], in_=y_t[:, :])

            def last_tile_split(r0):
                # Column-halved version of dma+dot+combine for the final tile:
                # shortens the critical chain after the last input byte lands.
                HF = D // 2
                x_t = xs.tile([P, D], dt)
                idma_eng.dma_start(out=x_t[:, 0:HF], in_=x_ext[r0 : r0 + P, 0:HF])
                idma_eng.dma_start(out=x_t[:, HF:], in_=x_ext[r0 : r0 + P, HF:])
                t1 = ts.tile([P, D], dt)
                t2 = ts.tile([P, D], dt, tag="t2")
                dotA = ss.tile([P, 1], f32, tag="dotA")
                dotB = ss.tile([P, 1], f32, tag="dotB")
                nc.vector.tensor_mul(t1[:, 0:HF], x_t[:, 0:HF], wd_t[:, 0:HF])
                nc.scalar.activation(
                    t2[:, 0:HF], t1[:, 0:HF],
                    mybir.ActivationFunctionType.Copy, accum_out=dotA[:, :])
                nc.vector.tensor_mul(t1[:, HF:], x_t[:, HF:], wd_t[:, HF:])
                nc.scalar.activation(
                    t2[:, HF:], t1[:, HF:],
                    mybir.ActivationFunctionType.Copy, accum_out=dotB[:, :])
                dotv = ss.tile([P, 1], f32)
                nc.vector.tensor_add(dotv[:, :], dotA[:, :], dotB[:, :])
                h = ss.tile([P, 1], f32)
                nc.scalar.activation(
                    h[:, :], dotv[:, :], mybir.ActivationFunctionType.Tanh,
                    bias=bias_t[:, :], scale=1.0)
                return x_t, h

            def combine_split(x_t, h, r0):
                HF = D // 2
                y_t = ys.tile([P, D], dt)
                y1 = ts.tile([P, D], dt, tag="y1")
                for c0, c1 in ((0, HF), (HF, D)):
                    nc.vector.tensor_scalar(
                        out=y1[:, c0:c1], in0=kw_t[:, c0:c1], scalar1=h[:, :],
                        scalar2=None, op0=mult)
                    if add_x:
                        nc.vector.tensor_add(
                            y_t[:, c0:c1], y1[:, c0:c1], x_t[:, c0:c1])
                    else:
                        nc.vector.tensor_copy(y_t[:, c0:c1], y1[:, c0:c1])
                    odma_eng.dma_start(
                        out=y_ext[r0 : r0 + P, c0:c1], in_=y_t[:, c0:c1])

            def dot_pass_halves(x_t):
                # Column-halved dot: each half's mult depends only on that
                # half of wd_t, so tile 0 can start before the wd broadcast
                # fully completes (Tile tracks slice-granular deps).
                HF = D // 2
                t1 = ts.tile([P, D], dt)
                t2 = ts.tile([P, D], dt, tag="t2")
                dotA = ss.tile([P, 1], f32, tag="dotA")
                dotB = ss.tile([P, 1], f32, tag="dotB")
                for (c0, c1), dv in (((0, HF), dotA), ((HF, D), dotB)):
                    nc.vector.tensor_mul(
                        t1[:, c0:c1], x_t[:, c0:c1], wd_t[:, c0:c1])
                    nc.scalar.activation(
                        t2[:, c0:c1], t1[:, c0:c1],
                        mybir.ActivationFunctionType.Copy, accum_out=dv[:, :])
                dotv = ss.tile([P, 1], f32)
                nc.vector.tensor_add(dotv[:, :], dotA[:, :], dotB[:, :])
                h = ss.tile([P, 1], f32)
                nc.scalar.activation(
                    h[:, :], dotv[:, :], mybir.ActivationFunctionType.Tanh,
                    bias=bias_t[:, :], scale=1.0)
                return h

            # Software-pipelined: the combine for tile i-LAG is emitted after
            # tile i's dot pass, so the DVE never waits on ACT's tanh.
            lag = int(os.environ.get("NK_LAG", "1"))
            split0 = os.environ.get("NK_SPLIT0", "0") == "1"
            pending = []
            for i in range(n_tiles - 1):
                r0 = i * P
                x_t = xs.tile([P, D], dt)
                idma_eng.dma_start(out=x_t[:, :], in_=x_ext[r0 : r0 + P, :])
                if i == 0 and split0 and const_bcast in ("pe", "kwpe"):
                    h = dot_pass_halves(x_t)
                else:
                    h = dot_pass(x_t)
                if i == 0 and const_bcast in ("pe", "kwpe"):
                    bcast(kw_row, kw_t)
                pending.append((x_t, h, r0))
                if len(pending) > lag:
                    combine(*pending.pop(0))
            r0_last = (n_tiles - 1) * P
            x_last, h_last = last_tile_split(r0_last)
            for args in pending:
                combine(*args)
            combine_split(x_last, h_last, r0_last)

    nc.compile()
    return nc


def _get_nc(add_x, b_val):
    key = (add_x, float(b_val))
    if key not in _CACHE:
        _CACHE[key] = _build(add_x, b_val)
    return _CACHE[key]


last_results = None


def kernel(x, w, c, k, b):
    import ml_dtypes
    from concourse.bass_utils import run_bass_kernel_spmd

    global last_results

    x = np.asarray(x, dtype=np.float32)
    w = np.asarray(w, dtype=np.float32).reshape(-1)
    c_val = float(np.asarray(c).reshape(-1)[0])
    k_val = float(np.asarray(k).reshape(-1)[0])
    b_val = float(np.asarray(b).reshape(-1)[0])
    assert x.shape == (B, D) and w.shape == (D,)

    add_x = c_val != 0.0
    if c_val not in (0.0, 1.0):
        x = c_val * x
        wd = w / (D * c_val)
    else:
        wd = w / D
    kw = k_val * w

    np_dt = ml_dtypes.bfloat16 if DTYPE == "bf16" else np.float32
    x_dev = x.astype(np_dt)
    if COMB == "pe2":
        wd_p, kw_p = P, 1
    else:
        wd_p = P if CONST_BCAST in ("0", "kwpe") else 1
        kw_p = P if CONST_BCAST == "0" else 1
    wd_b = np.ascontiguousarray(
        np.broadcast_to(wd.astype(np_dt)[None, :], (wd_p, D)))
    kw_b = np.ascontiguousarray(
        np.broadcast_to(kw.astype(np_dt)[None, :], (kw_p, D)))

    nc = _get_nc(add_x, b_val)

    in_maps = [
        {
            "x": np.ascontiguousarray(x_dev[i * B_SHARD : (i + 1) * B_SHARD]),
            "wd": wd_b,
            "kw": kw_b,
        }
        for i in range(N_CORES)
    ]

    trace = os.environ.get("BASS_KERNEL_TRACE", "0") == "1"
    res = run_bass_kernel_spmd(
        nc, in_maps, core_ids=list(range(N_CORES)), trace=trace
    )
    last_results = res
    y = np.concatenate([res.results[i]["y"] for i in range(N_CORES)], axis=0)
    return y.astype(np.float32)


# revision 45
# speedup vs baseline: 1.0234x; 1.0234x over previous
"""Trainium2 Bass kernel for: y = k*tanh(x@w/d + b)[:,None] * w[None,:] + c*x.

Data-parallel over 8 NeuronCores: x is [16384, 4096] f32, sharded 2048
rows/core; w/c/k/b are tiny and folded host-side:
  wd = w/d            (dot-product weights; /d folded in)
  kw = k*w            (outer-product weights; k folded in)
  b  -> tanh bias
  c  -> if c != 1: feed x' = c*x and wd' = w/(d*c); identity otherwise.

Per-core device program (16 tiles of [128 rows, 4096 cols], bf16 I/O):
  DMA in x_tile
  dot  = sum(x * wd) per row     (one fused DVE pass, or DVE mult + ACT accum)
  h    = tanh(dot + b)           (ACT)
  y    = kw * h + c*x            (DVE tensor_scalar + add, or TensorE PSUM
                                  accumulate: identity-matmul c*x + rank-1
                                  h (x) kw, with ACT PSUM->SBUF copy-out)
  DMA out y_tile

Memory-bound: ~32 MB HBM traffic/core at ~358 GB/s. bf16 I/O halves DMA
traffic and doubles DVE throughput at ~0.3% output error (well under the
tolerance); h is computed via an f32 accumulator.
"""

import os

import numpy as np

B = 16384
D = 4096
N_CORES = 8
P = 128
B_SHARD = B // N_CORES          # 2048 rows per core
N_TILES = B_SHARD // P          # 16 tiles per core

# variant knobs (resolved at import; see _build)
DTYPE = os.environ.get("NK_DTYPE", "bf16")        # "f32" | "bf16"
DOT = os.environ.get("NK_DOT", "tt_act")          # "stt" | "tt_act"
COMB = os.environ.get("NK_COMB", "split")         # "stt" | "split" | "split_gp" | "pe"
GP_COLS = int(os.environ.get("NK_GP_COLS", "0"))  # cols of final add on gpsimd
XBUFS = int(os.environ.get("NK_XBUFS", "8"))
YBUFS = int(os.environ.get("NK_YBUFS", "3"))
TSBUFS = int(os.environ.get("NK_TSBUFS", "2"))
CONST_BCAST = os.environ.get("NK_CONST_BCAST", "pe")  # "0" | "gp" | "pe"
ODMA = os.environ.get("NK_ODMA", "sync")          # engine issuing out-DMAs
# x in-DMAs issue from GPSIMD's queue so their prefetch is never blocked
# behind an out-DMA (on sync) waiting for its data dependency: -2.4us measured.
IDMA = os.environ.get("NK_IDMA", "gpsimd")

_CACHE = {}


def _build(add_x, b_val, dtype=DTYPE, dot=DOT, comb=COMB, gp_cols=GP_COLS,
           xbufs=XBUFS, ybufs=YBUFS, n_tiles=N_TILES, tsbufs=TSBUFS,
           const_bcast=CONST_BCAST, odma=ODMA):
    """Build + compile the per-core Bass program (SPMD, same graph on all cores)."""
    from contextlib import ExitStack

    import concourse.bass as bass  # noqa: F401
    import concourse.tile as tile
    from concourse import bacc, mybir

    f32 = mybir.dt.float32
    dt = mybir.dt.bfloat16 if dtype == "bf16" else f32
    rows = n_tiles * P

    nc = bacc.Bacc(
        "TRN2",
        debug=False,
        target_bir_lowering=False,
        num_devices=N_CORES,
    )

    # wd: full [P, D] direct DMA unless a broadcast mode covers it ("kwpe"
    # broadcasts only kw and DMAs wd directly — wd gates the first dot pass,
    # and the DMA lands sooner than the PE-broadcast chain completes).
    if comb == "pe2":
        # wd direct [P, D] DMA (frees all PSUM for the combine); kw as a row.
        wd_p, kw_p = P, 1
    else:
        wd_p = P if const_bcast in ("0", "kwpe") else 1
        kw_p = P if const_bcast == "0" else 1
    x_ext = nc.dram_tensor("x", [rows, D], dt, kind="ExternalInput").ap()
    wd_ext = nc.dram_tensor("wd", [wd_p, D], dt, kind="ExternalInput").ap()
    kw_ext = nc.dram_tensor("kw", [kw_p, D], dt, kind="ExternalInput").ap()
    y_ext = nc.dram_tensor("y", [rows, D], dt, kind="ExternalOutput").ap()

    mult = mybir.AluOpType.mult
    add = mybir.AluOpType.add

    with tile.TileContext(nc) as tc, ExitStack() as ctx:
        consts = ctx.enter_context(tc.tile_pool(name="consts", bufs=1))
        xs = ctx.enter_context(tc.tile_pool(name="xs", bufs=xbufs))
        ys = ctx.enter_context(tc.tile_pool(name="ys", bufs=ybufs))
        ts = ctx.enter_context(tc.tile_pool(name="ts", bufs=tsbufs))
        ss = ctx.enter_context(tc.tile_pool(name="ss", bufs=int(os.environ.get("NK_SSBUFS", "4"))))

        odma_eng = getattr(nc, odma)
        idma_eng = getattr(nc, IDMA)
        bias_t = None

        def make_bias():
            t = consts.tile([P, 1], f32, tag="bias")
            nc.gpsimd.memset(t[:, :], float(b_val))
            return t

        if comb == "pe2":
            # v2 of the TensorE combine: DVE does only the fused dot;
            # TensorE accumulates c*x (identity matmul) + h (x) kw (K=1
            # rank-1 matmul) into [P, 2048] PSUM chunks (4 banks, bufs=2 =
            # all 8 banks); ACT copies PSUM->SBUF. wd arrives as a direct
            # [P, D] DMA; kw stays a [1, D] row.
            assert add_x
            wd_t = consts.tile([P, D], dt)
            kw_row = consts.tile([1, D], dt, tag="kw_row")
            nc.sync.dma_start(out=wd_t[:, :], in_=wd_ext[:, :])
            nc.sync.dma_start(out=kw_row[:, :], in_=kw_ext[:, :])
            # identity (c folded host-side; c==1 here)
            ci_t = consts.tile([P, P], dt, tag="ci")
            iota_t = consts.tile([P, P], f32, tag="iota")
            nc.gpsimd.iota(
                iota_t[:, :], [[1, P]], channel_multiplier=-1,
                allow_small_or_imprecise_dtypes=True,
            )
            nc.vector.tensor_scalar(
                out=ci_t[:, :], in0=iota_t[:, :], scalar1=0.0, scalar2=None,
                op0=mybir.AluOpType.is_equal,
            )
            bias_t = make_bias()

            yps = ctx.enter_context(tc.tile_pool(name="yps", bufs=2, space="PSUM"))
            hs = ctx.enter_context(tc.tile_pool(name="hs", bufs=4))

            def dot_pass_pe(x_t):
                trash = ts.tile([P, D], dt)
                dotv = ss.tile([P, 1], f32)
                nc.vector.scalar_tensor_tensor(
                    out=trash[:, :], in0=x_t[:, :], scalar=1.0, in1=wd_t[:, :],
                    op0=mult, op1=mult, accum_out=dotv[:, :],
                )
                h = hs.tile([P, 1], dt, tag="h")
                nc.scalar.activation(
                    h[:, :], dotv[:, :], mybir.ActivationFunctionType.Tanh,
                    bias=bias_t[:, :], scale=1.0,
                )
                # transpose h [P,1] -> hT [1,P] with a tiny SBUF->SBUF DMA
                hT = hs.tile([1, P], dt, tag="hT")
                nc.sync.dma_start(out=hT[0:1, :], in_=h[:, 0:1])
                return hT

            def combine_pe(x_t, hT, r0):
                y_t = ys.tile([P, D], dt)
                for cchunk in range(D // 2048):
                    cs = cchunk * 2048
                    ps = yps.tile([P, 2048], f32, tag="y")
                    for s in range(4):
                        a = cs + s * 512
                        nc.tensor.matmul(ps[:, s * 512 : (s + 1) * 512],
                                         ci_t[:, :], x_t[:, a : a + 512],
                                         start=True, stop=False)
                    for s in range(4):
                        a = cs + s * 512
                        nc.tensor.matmul(ps[:, s * 512 : (s + 1) * 512],
                                         hT[:, :], kw_row[:, a : a + 512],
                                         start=False, stop=True)
                    nc.scalar.copy(y_t[:, cs : cs + 2048], ps[:, :])
                odma_eng.dma_start(out=y_ext[r0 : r0 + P, :], in_=y_t[:, :])

            prev = None
            for i in range(n_tiles):
                r0 = i * P
                x_t = xs.tile([P, D], dt)
                idma_eng.dma_start(out=x_t[:, :], in_=x_ext[r0 : r0 + P, :])
                hT = dot_pass_pe(x_t)
                if prev is not None:
                    combine_pe(*prev)
                prev = (x_t, hT, r0)
            combine_pe(*prev)

        elif comb == "pe":
            # DVE does only the fused dot; TensorE accumulates c*x + h (x) kw
            # in PSUM (identity matmul + K=1 rank-1 matmul); ACT copies
            # PSUM->SBUF. kw stays a [1, D] row (rank-1 rhs); wd is broadcast
            # to [P, D] via a ones-matmul on the otherwise idle TensorE.
            assert const_bcast == "pe" and add_x
            wd_t = consts.tile([P, D], dt)
            wd_row = consts.tile([1, D], dt, tag="wd_row")
            kw_row = consts.tile([1, D], dt, tag="kw_row")
            ones_t = consts.tile([1, P], dt, tag="ones")
            nc.sync.dma_start(out=wd_row[:, :], in_=wd_ext[:, :])
            nc.sync.dma_start(out=kw_row[:, :], in_=kw_ext[:, :])
            nc.gpsimd.memset(ones_t[:, :], 1.0)
            bc_psum = ctx.enter_context(
                tc.tile_pool(name="bc_psum", bufs=2, space="PSUM"))
            for cchunk in range(D // 512):
                cs = cchunk * 512
                ps = bc_psum.tile([P, 512], f32, tag="bc")
                nc.tensor.matmul(
                    ps[:, :], ones_t[:, :], wd_row[:, cs : cs + 512],
                    start=True, stop=True,
                )
                nc.scalar.copy(wd_t[:, cs : cs + 512], ps[:, :])
            # identity (times c, folded host-side into x already; c==1 here)
            ci_t = consts.tile([P, P], dt, tag="ci")
            iota_t = consts.tile([P, P], f32, tag="iota")
            nc.gpsimd.iota(
                iota_t[:, :], [[1, P]], channel_multiplier=-1,
                allow_small_or_imprecise_dtypes=True,
            )
            nc.vector.tensor_scalar(
                out=ci_t[:, :], in0=iota_t[:, :], scalar1=0.0, scalar2=None,
                op0=mybir.AluOpType.is_equal,
            )
            bias_t = make_bias()

            yps = ctx.enter_context(tc.tile_pool(name="yps", bufs=3, space="PSUM"))
            hs = ctx.enter_context(tc.tile_pool(name="hs", bufs=4))

            def dot_pass_pe(x_t):
                trash = ts.tile([P, D], dt)
                dotv = ss.tile([P, 1], f32)
                nc.vector.scalar_tensor_tensor(
                    out=trash[:, :], in0=x_t[:, :], scalar=1.0, in1=wd_t[:, :],
                    op0=mult, op1=mult, accum_out=dotv[:, :],
                )
                h = hs.tile([P, 1], dt, tag="h")
                nc.scalar.activation(
                    h[:, :], dotv[:, :], mybir.ActivationFunctionType.Tanh,
                    bias=bias_t[:, :], scale=1.0,
                )
                # transpose h [P,1] -> hT [1,P] with a tiny SBUF->SBUF DMA
                hT = hs.tile([1, P], dt, tag="hT")
                nc.sync.dma_start(out=hT[0:1, :], in_=h[:, 0:1])
                return hT

            def combine_pe(x_t, hT, r0):
                y_t = ys.tile([P, D], dt)
                pss = []
                for cchunk in range(D // 1024):
                    cs = cchunk * 1024
                    ps = yps.tile([P, 1024], f32, tag="y")
                    nc.tensor.matmul(ps[:, 0:512], ci_t[:, :],
                                     x_t[:, cs : cs + 512], start=True, stop=False)
                    nc.tensor.matmul(ps[:, 512:1024], ci_t[:, :],
                                     x_t[:, cs + 512 : cs + 1024], start=True, stop=False)
                    pss.append((cs, ps))
                for cs, ps in pss:
                    nc.tensor.matmul(ps[:, 0:512], hT[:, :],
                                     kw_row[:, cs : cs + 512], start=False, stop=True)
                    nc.tensor.matmul(ps[:, 512:1024], hT[:, :],
                                     kw_row[:, cs + 512 : cs + 1024], start=False, stop=True)
                for cs, ps in pss:
                    nc.scalar.copy(y_t[:, cs : cs + 1024], ps[:, :])
                odma_eng.dma_start(out=y_ext[r0 : r0 + P, :], in_=y_t[:, :])

            prev = None
            for i in range(n_tiles):
                r0 = i * P
                x_t = xs.tile([P, D], dt)
                idma_eng.dma_start(out=x_t[:, :], in_=x_ext[r0 : r0 + P, :])
                hT = dot_pass_pe(x_t)
                if prev is not None:
                    combine_pe(*prev)
                prev = (x_t, hT, r0)
            combine_pe(*prev)

        else:
            wd_t = consts.tile([P, D], dt)
            kw_t = consts.tile([P, D], dt)
            if const_bcast == "gp":
                wd_row = consts.tile([1, D], dt, tag="wd_row")
                kw_row = consts.tile([1, D], dt, tag="kw_row")
                nc.sync.dma_start(out=wd_row[:, :], in_=wd_ext[:, :])
                nc.sync.dma_start(out=kw_row[:, :], in_=kw_ext[:, :])
                nc.gpsimd.partition_broadcast(wd_t[:, :], wd_row[:, :])
                nc.gpsimd.partition_broadcast(kw_t[:, :], kw_row[:, :])
            elif const_bcast in ("pe", "kwpe"):
                kw_row = consts.tile([1, D], dt, tag="kw_row")
                ones_t = consts.tile([1, P], dt, tag="ones")
                if const_bcast == "kwpe":
                    nc.sync.dma_start(out=wd_t[:, :], in_=wd_ext[:, :])
                else:
                    wd_row = consts.tile([1, D], dt, tag="wd_row")
                    nc.sync.dma_start(out=wd_row[:, :], in_=wd_ext[:, :])
                nc.sync.dma_start(out=kw_row[:, :], in_=kw_ext[:, :])
                nc.gpsimd.memset(ones_t[:, :], 1.0)
                psum = ctx.enter_context(
                    tc.tile_pool(name="bc_psum", bufs=2, space="PSUM"))

                bc_w = int(os.environ.get("NK_BC_W", "1024"))

                def bcast(row, dst, split_engines=False):
                    # bc_w-wide chunks: bc_w/512 matmuls (one PSUM bank each)
                    # + 1 PSUM->SBUF copy. split_engines alternates the copies
                    # between ACT and DVE (measured worse — keep off).
                    for cchunk in range(D // bc_w):
                        cs = cchunk * bc_w
                        ps = psum.tile([P, bc_w], f32, tag="bc")
                        for s in range(bc_w // 512):
                            a = cs + s * 512
                            nc.tensor.matmul(
                                ps[:, s * 512 : (s + 1) * 512], ones_t[:, :],
                                row[:, a : a + 512], start=True, stop=True,
                            )
                        if split_engines and cchunk % 2 == 1:
                            nc.vector.tensor_copy(dst[:, cs : cs + bc_w], ps[:, :])
                        else:
                            nc.scalar.copy(dst[:, cs : cs + bc_w], ps[:, :])

                # wd is needed by the very first dot pass -> broadcast it now
                # (unless it came via direct DMA in "kwpe" mode); kw is first
                # needed ~10us later and is broadcast after tile 0's dot pass.
                if const_bcast == "pe":
                    bcast(wd_row, wd_t)
            else:
                nc.sync.dma_start(out=wd_t[:, :], in_=wd_ext[:, :])
                nc.sync.dma_start(out=kw_t[:, :], in_=kw_ext[:, :])
            bias_t = make_bias()

            def dot_pass(x_t):
                dotv = ss.tile([P, 1], f32)
                if dot == "stt":
                    trash = ts.tile([P, D], dt)
                    nc.vector.scalar_tensor_tensor(
                        out=trash[:, :], in0=x_t[:, :], scalar=1.0, in1=wd_t[:, :],
                        op0=mult, op1=mult, accum_out=dotv[:, :],
                    )
                else:  # tt_act: DVE multiply, ACT accumulate-copy
                    t1 = ts.tile([P, D], dt)
                    nc.vector.tensor_mul(t1[:, :], x_t[:, :], wd_t[:, :])
                    t2 = ts.tile([P, D], dt, tag="t2")
                    nc.scalar.activation(
                        t2[:, :], t1[:, :], mybir.ActivationFunctionType.Copy,
                        accum_out=dotv[:, :],
                    )
                h = ss.tile([P, 1], f32)
                nc.scalar.activation(
                    h[:, :], dotv[:, :], mybir.ActivationFunctionType.Tanh,
                    bias=bias_t[:, :], scale=1.0,
                )
                return h

            def combine(x_t, h, r0):
                y_t = ys.tile([P, D], dt)
                if comb == "stt":
                    nc.vector.scalar_tensor_tensor(
                        out=y_t[:, :], in0=kw_t[:, :], scalar=h[:, :], in1=x_t[:, :],
                        op0=mult, op1=add if add_x else mybir.AluOpType.bypass,
                    )
                elif comb == "split_ip":
                    # in-place: y = kw*h, then y += x (no y1 buffer)
                    nc.vector.tensor_scalar(
                        out=y_t[:, :], in0=kw_t[:, :], scalar1=h[:, :], scalar2=None,
                        op0=mult,
                    )
                    if add_x:
                        nc.vector.tensor_add(y_t[:, :], y_t[:, :], x_t[:, :])
                else:
                    y1 = ts.tile([P, D], dt, tag="y1")
                    act_cols = int(os.environ.get("NK_ACT_COLS", "0"))
                    if comb == "split_act" and act_cols > 0:
                        # ACT computes a column-slice of kw*h via
                        # out = Copy(in*scale) with per-partition scale=h,
                        # offloading part of the DVE tensor_scalar.
                        nc.scalar.activation(
                            y1[:, :act_cols], kw_t[:, :act_cols],
                            mybir.ActivationFunctionType.Copy,
                            bias=0.0, scale=h[:, :],
                        )
                        nc.vector.tensor_scalar(
                            out=y1[:, act_cols:], in0=kw_t[:, act_cols:],
                            scalar1=h[:, :], scalar2=None, op0=mult,
                        )
                    else:
                        nc.vector.tensor_scalar(
                            out=y1[:, :], in0=kw_t[:, :], scalar1=h[:, :],
                            scalar2=None, op0=mult,
                        )
                    if not add_x:
                        y_t = y1
                    elif comb == "split_gp" and gp_cols > 0:
                        cs = D - gp_cols
                        nc.vector.tensor_add(y_t[:, :cs], y1[:, :cs], x_t[:, :cs])
                        nc.gpsimd.tensor_add(y_t[:, cs:], y1[:, cs:], x_t[:, cs:])
                    else:
                        nc.vector.tensor_add(y_t[:, :], y1[:, :], x_t[:, :])
                odma_eng.dma_start(out=y_ext[r0 : r0 + P, :], in_=y_t[:, :])

            def last_tile_split(r0):
                # Column-halved version of dma+dot+combine for the final tile:
                # shortens the critical chain after the last input byte lands.
                HF = D // 2
                x_t = xs.tile([P, D], dt)
                idma_eng.dma_start(out=x_t[:, 0:HF], in_=x_ext[r0 : r0 + P, 0:HF])
                idma_eng.dma_start(out=x_t[:, HF:], in_=x_ext[r0 : r0 + P, HF:])
                t1 = ts.tile([P, D], dt)
                t2 = ts.tile([P, D], dt, tag="t2")
                dotA = ss.tile([P, 1], f32, tag="dotA")
                dotB = ss.tile([P, 1], f32, tag="dotB")
                nc.vector.tensor_mul(t1[:, 0:HF], x_t[:, 0:HF], wd_t[:, 0:HF])
                nc.scalar.activation(
                    t2[:, 0:HF], t1[:, 0:HF],
                    mybir.ActivationFunctionType.Copy, accum_out=dotA[:, :])
                nc.vector.tensor_mul(t1[:, HF:], x_t[:, HF:], wd_t[:, HF:])
                nc.scalar.activation(
                    t2[:, HF:], t1[:, HF:],
                    mybir.ActivationFunctionType.Copy, accum_out=dotB[:, :])
                dotv = ss.tile([P, 1], f32)
                nc.vector.tensor_add(dotv[:, :], dotA[:, :], dotB[:, :])
                h = ss.tile([P, 1], f32)
                nc.scalar.activation(
                    h[:, :], dotv[:, :], mybir.ActivationFunctionType.Tanh,
                    bias=bias_t[:, :], scale=1.0)
                return x_t, h

            def combine_split(x_t, h, r0):
                HF = D // 2
                y_t = ys.tile([P, D], dt)
                y1 = ts.tile([P, D], dt, tag="y1")
                for c0, c1 in ((0, HF), (HF, D)):
                    nc.vector.tensor_scalar(
                        out=y1[:, c0:c1], in0=kw_t[:, c0:c1], scalar1=h[:, :],
                        scalar2=None, op0=mult)
                    if add_x:
                        nc.vector.tensor_add(
                            y_t[:, c0:c1], y1[:, c0:c1], x_t[:, c0:c1])
                    else:
                        nc.vector.tensor_copy(y_t[:, c0:c1], y1[:, c0:c1])
                    odma_eng.dma_start(
                        out=y_ext[r0 : r0 + P, c0:c1], in_=y_t[:, c0:c1])

            def dot_pass_halves(x_t):
                # Column-halved dot: each half's mult depends only on that
                # half of wd_t, so tile 0 can start before the wd broadcast
                # fully completes (Tile tracks slice-granular deps).
                HF = D // 2
                t1 = ts.tile([P, D], dt)
                t2 = ts.tile([P, D], dt, tag="t2")
                dotA = ss.tile([P, 1], f32, tag="dotA")
                dotB = ss.tile([P, 1], f32, tag="dotB")
                for (c0, c1), dv in (((0, HF), dotA), ((HF, D), dotB)):
                    nc.vector.tensor_mul(
                        t1[:, c0:c1], x_t[:, c0:c1], wd_t[:, c0:c1])
                    nc.scalar.activation(
                        t2[:, c0:c1], t1[:, c0:c1],
                        mybir.ActivationFunctionType.Copy, accum_out=dv[:, :])
                dotv = ss.tile([P, 1], f32)
                nc.vector.tensor_add(dotv[:, :], dotA[:, :], dotB[:, :])
                h = ss.tile([P, 1], f32)
                nc.scalar.activation(
                    h[:, :], dotv[:, :], mybir.ActivationFunctionType.Tanh,
                    bias=bias_t[:, :], scale=1.0)
                return h

            # Software-pipelined: the combine for tile i-LAG is emitted after
            # tile i's dot pass, so the DVE never waits on ACT's tanh.
            lag = int(os.environ.get("NK_LAG", "1"))
            split0 = os.environ.get("NK_SPLIT0", "0") == "1"
            pending = []
            for i in range(n_tiles - 1):
                r0 = i * P
                x_t = xs.tile([P, D], dt)
                idma_eng.dma_start(out=x_t[:, :], in_=x_ext[r0 : r0 + P, :])
                if i == 0 and split0 and const_bcast in ("pe", "kwpe"):
                    h = dot_pass_halves(x_t)
                else:
                    h = dot_pass(x_t)
                if i == 0 and const_bcast in ("pe", "kwpe"):
                    bcast(kw_row, kw_t)
                pending.append((x_t, h, r0))
                if len(pending) > lag:
                    combine(*pending.pop(0))
            r0_last = (n_tiles - 1) * P
            x_last, h_last = last_tile_split(r0_last)
            for args in pending:
                combine(*args)
            combine_split(x_last, h_last, r0_last)

    nc.compile()
    return nc


def _get_nc(add_x, b_val):
    key = (add_x, float(b_val))
    if key not in _CACHE:
        _CACHE[key] = _build(add_x, b_val)
    return _CACHE[key]


last_results = None


def kernel(x, w, c, k, b):
    import ml_dtypes
    from concourse.bass_utils import run_bass_kernel_spmd

    global last_results

    x = np.asarray(x, dtype=np.float32)
    w = np.asarray(w, dtype=np.float32).reshape(-1)
    c_val = float(np.asarray(c).reshape(-1)[0])
    k_val = float(np.asarray(k).reshape(-1)[0])
    b_val = float(np.asarray(b).reshape(-1)[0])
    assert x.shape == (B, D) and w.shape == (D,)

    add_x = c_val != 0.0
    if c_val not in (0.0, 1.0):
        x = c_val * x
        wd = w / (D * c_val)
    else:
        wd = w / D
    kw = k_val * w

    np_dt = ml_dtypes.bfloat16 if DTYPE == "bf16" else np.float32
    x_dev = x.astype(np_dt)
    if COMB == "pe2":
        wd_p, kw_p = P, 1
    else:
        wd_p = P if CONST_BCAST in ("0", "kwpe") else 1
        kw_p = P if CONST_BCAST == "0" else 1
    wd_b = np.ascontiguousarray(
        np.broadcast_to(wd.astype(np_dt)[None, :], (wd_p, D)))
    kw_b = np.ascontiguousarray(
        np.broadcast_to(kw.astype(np_dt)[None, :], (kw_p, D)))

    nc = _get_nc(add_x, b_val)

    in_maps = [
        {
            "x": np.ascontiguousarray(x_dev[i * B_SHARD : (i + 1) * B_SHARD]),
            "wd": wd_b,
            "kw": kw_b,
        }
        for i in range(N_CORES)
    ]

    trace = os.environ.get("BASS_KERNEL_TRACE", "0") == "1"
    res = run_bass_kernel_spmd(
        nc, in_maps, core_ids=list(range(N_CORES)), trace=trace
    )
    last_results = res
    y = np.concatenate([res.results[i]["y"] for i in range(N_CORES)], axis=0)
    return y.astype(np.float32)
